# revision 36
# baseline (speedup 1.0000x reference)
"""Trainium2 Bass kernel for the capsule-routing layer.

Math (derived from the reference):
  u_hat[b,i,j,k] = sum_d x[b,j,d] W[d, i*32+k]   (never materialized!)
  iter t: c = softmax_i(b_logits); s[i,k] = sum_j c[i,j] u_hat[i,j,k]
          o = s / sqrt(sum_k s^2 + eps); b_logits[i,j] = sum_k o[i,k] u_hat[i,j,k]
Substituting u_hat = x @ W everywhere:
  y[i,d]   = sum_j c[i,j] x[j,d]            (small matmul, K=1024)
  sT[ik,i] = sum_d W[d,ik] yT[d,i]          (W stationary -> s lands transposed)
  wtil[d,i]= sum_ik WT[ik,d] maskT*sT       (WT stationary -> wtil lands [d, i])
  b[i,j]   = sum_d wtil[d,i] x[j,d]         (small matmul, K=256)
  exp(rn*b) folds the squash scale into the softmax numerator (softmax of
  rn*b equals softmax-of-(o . u_hat)) since rn is constant along j.
This removes the 34-GFLOP u_hat product entirely (~7.6x FLOP reduction), and
the transposed-s / transposed-wtil orientations remove all DMA transposes
from the s -> wtil -> b chain (only the e-transpose remains).

The capsule norm (sum_k s^2) in transposed space is a partition reduction:
square on ACT, ones-matmul on PE (replicated to 32 rows), then 4 DVE 32x32
stream-transposes flip the [1,128] row into the [128,1] per-partition scalar
that exp(scale=rn) consumes. The whole norm branch runs concurrently with
the wtil/b matmuls.

Sharding: data-parallel, 8 batches per core; batches processed in groups of
4 stacked on SBUF partitions (partition p = 32*b + i).
"""

import numpy as np

try:
    import concourse.bass as bass
except ImportError:  # path fallback for bare environments
    import sys

    sys.path.insert(0, "/opt/trn_rl_repo")
    import concourse.bass as bass

from contextlib import ExitStack

import concourse.bacc as bacc
import concourse.tile as tile
from concourse import mybir
from concourse.bass_utils import run_bass_kernel_spmd

F32 = mybir.dt.float32
F32R = mybir.dt.float32r
BF16 = mybir.dt.bfloat16
AF = mybir.ActivationFunctionType
ALU = mybir.AluOpType

NUM_CAPS = 32
DIM_CAPS = 32
D_IN = 256  # feature dim (d)
N_IN = 1024  # input capsule count (j)
IK = NUM_CAPS * DIM_CAPS  # 1024 flattened (i,k)
B_TOTAL = 64
N_CORES = 8
B_PER_CORE = 8
GB = 4  # batches per partition-group
GROUPS = B_PER_CORE // GB  # 2
EPS = 1e-7
ROUTINGS = 3

import os as _os

DEBUG = _os.environ.get("K_DEBUG", "0") == "1"
USE_GS = _os.environ.get("K_GS", "1") == "1"  # softmax reduce/mult on GpSimd
NEWTON = int(_os.environ.get("K_NEWTON", "2"))


def build_program():
    nc = bacc.Bacc("TRN2", target_bir_lowering=False, debug=False)

    x_b = nc.declare_dram_parameter("x_b", [B_PER_CORE, N_IN, D_IN], BF16, isOutput=False)
    x_d = nc.declare_dram_parameter("x_d", [B_PER_CORE, D_IN, N_IN], BF16, isOutput=False)
    w_a = nc.declare_dram_parameter("w_a", [D_IN, IK], BF16, isOutput=False)
    w_t = nc.declare_dram_parameter("w_t", [IK, D_IN], BF16, isOutput=False)
    maskt_d = nc.declare_dram_parameter("maskt", [128, 8, 128], BF16, isOutput=False)
    sel_d = nc.declare_dram_parameter("sel", [128, DIM_CAPS], BF16, isOutput=False)
    ident_d = nc.declare_dram_parameter("ident", [128, 128], F32, isOutput=False)
    # transposed output: out[g, k, (b,i)]
    out_d = nc.declare_dram_parameter("out", [GROUPS, DIM_CAPS, 128], F32, isOutput=True)
    if DEBUG:
        dbg_m4t = nc.declare_dram_parameter("dbg_m4t", [GROUPS, 2, 128, 8, 128], BF16, isOutput=True)
        dbg_nrm = nc.declare_dram_parameter("dbg_nrm", [GROUPS, 2, 128, 1], F32, isOutput=True)
        dbg_wt = nc.declare_dram_parameter("dbg_wt", [GROUPS, 2, 128, 2, 128], BF16, isOutput=True)
        dbg_e4 = nc.declare_dram_parameter("dbg_e4", [GROUPS, 2, 128, 8, 128], BF16, isOutput=True)
        dbg_ct = nc.declare_dram_parameter("dbg_ct", [GROUPS, 2, 128, 8, 128], BF16, isOutput=True)

    with ExitStack() as ctx:
        tc = ctx.enter_context(tile.TileContext(nc))
        singles = ctx.enter_context(tc.tile_pool(name="singles", bufs=1))
        xpool = ctx.enter_context(tc.tile_pool(name="xpool", bufs=8))
        work = ctx.enter_context(tc.tile_pool(name="work", bufs=3))
        psum = ctx.enter_context(tc.tile_pool(name="ps", bufs=1, space="PSUM"))

        # ---- static tensors ----
        # SP (sync) DMA queue carries only what gates the first compute steps
        # (w_a, ident) and then the x tiles in need-order; everything else
        # rides the Activation HWDGE queue, which is idle early.
        w_a_sb = singles.tile([128, 2, IK], BF16)  # [d%128, d//128, (ik)]
        nc.sync.dma_start(out=w_a_sb[:, :, :], in_=w_a[:, :].rearrange("(c p) n -> p c n", p=128))
        ident_sb = singles.tile([128, 128], F32)
        nc.sync.dma_start(out=ident_sb[:, :], in_=ident_d[:, :])
        w_t_sb = singles.tile([128, 8, D_IN], BF16)  # [(ik)%128, (ik)//128, d]
        nc.scalar.dma_start(out=w_t_sb[:, :, :], in_=w_t[:, :].rearrange("(c p) n -> p c n", p=128))
        maskt_sb = singles.tile([128, 8, 128], BF16)
        nc.scalar.dma_start(out=maskt_sb[:, :, :], in_=maskt_d[:, :, :])
        cu_sb = singles.tile([128, NUM_CAPS], BF16)
        nc.vector.memset(cu_sb[:, :], 1.0 / NUM_CAPS)
        ones_sb = singles.tile([128, NUM_CAPS], F32)
        nc.vector.memset(ones_sb[:, :], 1.0)
        magic_sb = singles.tile([128, 1], mybir.dt.int32)
        nc.vector.memset(magic_sb[:, :], 0x5F3759DF)
        one_i_sb = singles.tile([128, 1], mybir.dt.int32)
        nc.vector.memset(one_i_sb[:, :], 1)
        # sel32[p, m] = 1 iff p % 32 == m: partition-strided k-group reducer
        sel_sb = singles.tile([128, DIM_CAPS], BF16)
        nc.scalar.dma_start(out=sel_sb[:, :], in_=sel_d[:, :])

        def rsqrt_dve(a_ap, tagp):
            """1/sqrt(a): DVE quake bit-trick + Newton iterations."""
            t_i = work.tile([128, 1], mybir.dt.int32, tag=tagp + "i", name="nr_i")
            nc.vector.tensor_tensor(
                t_i[:, :], a_ap.bitcast(mybir.dt.int32), one_i_sb[:, :], ALU.logical_shift_right
            )
            r = work.tile([128, 1], F32, tag=tagp + "r", name="nr_r")
            nc.vector.tensor_tensor(
                r[:, :].bitcast(mybir.dt.int32), magic_sb[:, :], t_i[:, :], ALU.subtract
            )
            t2 = work.tile([128, 1], F32, tag=tagp + "t", name="nr_t")
            for _ in range(NEWTON):
                nc.vector.tensor_mul(t2[:, :], a_ap, r[:, :])
                nc.vector.tensor_mul(t2[:, :], t2[:, :], r[:, :])
                nc.vector.tensor_scalar(t2[:, :], t2[:, :], -0.5, 1.5, ALU.mult, ALU.add)
                nc.vector.tensor_mul(r[:, :], r[:, :], t2[:, :])
            return r

        def rsqrt_row(a_ap, tagp):
            """1/sqrt(a) for [32, 128] row tiles (quake + Newton)."""
            t_i = work.tile([32, 128], mybir.dt.int32, tag=tagp + "ri", name="nw_i")
            nc.vector.tensor_tensor(
                t_i[:, :],
                a_ap.bitcast(mybir.dt.int32),
                one_i_sb[0:32, 0:1].broadcast_to([32, 128]),
                ALU.logical_shift_right,
            )
            r = work.tile([32, 128], F32, tag=tagp + "rr", name="nw_r")
            nc.vector.tensor_tensor(
                r[:, :].bitcast(mybir.dt.int32),
                magic_sb[0:32, 0:1].broadcast_to([32, 128]),
                t_i[:, :],
                ALU.subtract,
            )
            t2 = work.tile([32, 128], F32, tag=tagp + "rt", name="nw_t")
            for _ in range(NEWTON):
                nc.vector.tensor_mul(t2[:, :], a_ap, r[:, :])
                nc.vector.tensor_mul(t2[:, :], t2[:, :], r[:, :])
                nc.vector.tensor_scalar(t2[:, :], t2[:, :], -0.5, 1.5, ALU.mult, ALU.add)
                nc.vector.tensor_mul(r[:, :], r[:, :], t2[:, :])
            return r

        def group_stream(g):
            # ---- load this group's x in both layouts ----
            xb_t = []
            xd_t = []
            for b in range(GB):
                bb = g * GB + b
                xb = xpool.tile([128, 8, D_IN], BF16, tag="xb", name=f"xb{bb}")
                nc.sync.dma_start(out=xb[:, :, :], in_=x_b[bb].rearrange("(c p) n -> p c n", p=128))
                xb_t.append(xb)
            xd_eng = nc.scalar if g == 1 else nc.sync
            for b in range(GB):
                bb = g * GB + b
                xd = xpool.tile([128, 2, N_IN], BF16, tag="xd", name=f"xd{bb}")
                xd_eng.dma_start(out=xd[:, :, :], in_=x_d[bb].rearrange("(c p) n -> p c n", p=128))
                xd_t.append(xd)
            yield

            cT_sb = None  # [j%128, j//128, (4b,32i)] softmax'd coupling coeffs
            for it in range(ROUTINGS):
                last = it == ROUTINGS - 1

                # ---- y-MM: y[b,i,d] = sum_j c[b,i,j] x[b,j,d] ----
                # iter 0 runs batch-serial so compute starts as soon as the
                # first x tile lands (DMA-gated phase); later iters use the
                # 4-way col-tiled concurrent order.
                y4_ps = psum.tile([128, 2, 128], F32, tag="m32", bufs=2, name="y4_ps")
                loop = (
                    [(jc, b) for b in range(GB) for jc in range(8)]
                    if it == 0
                    else [(jc, b) for jc in range(8) for b in range(GB)]
                )
                for jc, b in loop:
                    lhsT = cu_sb[:, :] if it == 0 else cT_sb[:, jc, 32 * b : 32 * b + 32]
                    nc.tensor.matmul(
                        y4_ps[32 * b : 32 * b + 32, :, :].rearrange("p c n -> p (c n)"),
                        lhsT,
                        xb_t[b][:, jc, :],
                        start=(jc == 0),
                        stop=(jc == 7),
                        tile_position=(0, 32 * b),
                        skip_group_check=True,
                    )
                yield

                # evacuate + transpose y -> [d, (4b,32i)]
                y4_sb = work.tile([128, D_IN], F32, tag="y4sb", name="y4_sb")
                nc.scalar.copy(y4_sb[:, :], y4_ps[:, :, :].rearrange("p c n -> p (c n)"))
                yT_ps = psum.tile([128, 2, 128], F32, tag="m32", bufs=2, name="yT_ps")
                for t in range(2):
                    nc.tensor.transpose(yT_ps[:, t, :], y4_sb[:, 128 * t : 128 * t + 128], ident_sb[:, :])
                yT_sb = work.tile([128, 2, 128], BF16, tag="yTsb", name="yT_sb")
                nc.scalar.copy(yT_sb[:, :, :], yT_ps[:, :, :])
                yield

                if last:
                    # ---- last iter, fully in transposed space ----
                    # sT-MM + mask as in the main loop, then a selector-matmul
                    # (sel32[p,m]=1 iff p%32==m) does the per-k compaction as a
                    # partition-strided reduce on the PE:
                    #   scT[k, (b,i)] = sum_ikc sum_{p%32==k} m4T[p, ikc, (b,i)]
                    sT_ps = psum.tile([128, 8, 128], F32, tag="big", bufs=2, name="sT_ps")
                    m4t_sb = work.tile([128, 8, 128], BF16, tag="m4t", name="m4t_sb")
                    for ikc in range(8):
                        for dc in range(2):
                            nc.tensor.matmul(
                                sT_ps[:, ikc, :],
                                w_a_sb[:, dc, 128 * ikc : 128 * ikc + 128],
                                yT_sb[:, dc, :],
                                start=(dc == 0),
                                stop=(dc == 1),
                                skip_group_check=True,
                            )
                    for h in range(2):
                        hs = slice(4 * h, 4 * h + 4)
                        nc.vector.tensor_mul(m4t_sb[:, hs, :], sT_ps[:, hs, :], maskt_sb[:, hs, :])
                    scT_ps = psum.tile([32, 128], F32, tag="nrm", bufs=2, name="scT_ps")
                    for ikc in range(8):
                        nc.tensor.matmul(
                            scT_ps[:, :],
                            sel_sb[:, :],
                            m4t_sb[:, ikc, :],
                            start=(ikc == 0),
                            stop=(ikc == 7),
                            skip_group_check=True,
                        )
                    # norm rows: nr[*, (b,i)] = sum_k scT^2, replicated to 32 rows
                    sq_sb = work.tile([32, 128], F32, tag="lsq", name="lsq")
                    nc.scalar.activation(sq_sb[:, :], scT_ps[:, :], AF.Square)
                    nrw_ps = psum.tile([32, 128], F32, tag="nrm", bufs=2, name="nrw_ps")
                    nc.tensor.matmul(
                        nrw_ps[:, :],
                        ones_sb[0:32, :],
                        sq_sb[:, :],
                        start=True,
                        stop=True,
                        skip_group_check=True,
                    )
                    nre = work.tile([32, 128], F32, tag="lne", name="lne")
                    nc.vector.tensor_scalar(nre[:, :], nrw_ps[:, :], EPS, None, ALU.add)
                    rnr = rsqrt_row(nre[:, :], "lst")
                    o_outT = work.tile([32, 128], F32, tag="ooT", name="o_outT")
                    nc.vector.tensor_tensor(o_outT[:, :], scT_ps[:, :], rnr[:, :], ALU.mult)
                    nc.scalar.dma_start(out=out_d[g], in_=o_outT[:, :])
                    return

                # ---- sT-MM: sT[(ik),(b,i)] = sum_d W[d,(ik)] yT[d,(b,i)] ----
                # W chunks are the stationary operand (bf16 -> FWL), s lands
                # pre-transposed so no DMA transpose is needed before wtil.
                sT_ps = psum.tile([128, 8, 128], F32, tag="big", bufs=2, name="sT_ps")
                m4t_sb = work.tile([128, 8, 128], BF16, tag="m4t", name="m4t_sb")
                sqt_sb = work.tile([128, 8, 128], F32, tag="sqt", name="sqt_sb")
                for ikc in range(8):
                    for dc in range(2):
                        nc.tensor.matmul(
                            sT_ps[:, ikc, :],
                            w_a_sb[:, dc, 128 * ikc : 128 * ikc + 128],
                            yT_sb[:, dc, :],
                            start=(dc == 0),
                            stop=(dc == 1),
                            skip_group_check=True,
                        )
                for h in range(2):
                    hs = slice(4 * h, 4 * h + 4)
                    nc.vector.tensor_mul(m4t_sb[:, hs, :], sT_ps[:, hs, :], maskt_sb[:, hs, :])
                    nc.scalar.activation(sqt_sb[:, hs, :], m4t_sb[:, hs, :], AF.Square)
                yield

                # ---- capsule norm: partition-reduce sum of squares via ones-MM ----
                nrm_ps = psum.tile([32, 128], F32, tag="nrm", bufs=2, name="nrm_ps")
                for ikc in range(8):
                    nc.tensor.matmul(
                        nrm_ps[:, :],
                        ones_sb[:, :],
                        sqt_sb[:, ikc, :],
                        start=(ikc == 0),
                        stop=(ikc == 7),
                        skip_group_check=True,
                    )
                # flip [1,128] norm row -> [128,1] per-partition scalar (DVE 32x32)
                nflip = work.tile([128, 32], F32, tag="nflip", name="nflip")
                for q in range(4):
                    nc.vector.transpose(nflip[32 * q : 32 * q + 32, :], nrm_ps[0:32, 32 * q : 32 * q + 32])
                nsq4 = work.tile([128, 1], F32, tag="nsq4", name="nsq4")
                nc.vector.tensor_scalar(nsq4[:, :], nflip[:, 0:1], EPS, None, ALU.add)
                rn4 = rsqrt_dve(nsq4[:, :], "mid")
                if DEBUG:
                    nc.sync.dma_start(out=dbg_m4t[g, it], in_=m4t_sb[:, :, :])
                    nc.sync.dma_start(out=dbg_nrm[g, it], in_=nsq4[:, :])
                yield

                # ---- wtil-MM: wtil[d,(b,i)] = sum_ik WT[(ik),d] m4T[(ik),(b,i)] ----
                # NB: accumulation must be contiguous per region — start=True
                # clears has_written for the whole bank, so interleaving two
                # regions' groups in one bank corrupts the first region.
                wt_ps = psum.tile([128, 2, 128], F32, tag="m32", bufs=2, name="wt_ps")
                for dh in range(2):
                    for ikc in range(8):
                        nc.tensor.matmul(
                            wt_ps[:, dh, :],
                            w_t_sb[:, ikc, 128 * dh : 128 * dh + 128],
                            m4t_sb[:, ikc, :],
                            start=(ikc == 0),
                            stop=(ikc == 7),
                            skip_group_check=True,
                        )
                wt_sb = work.tile([128, 2, 128], BF16, tag="wtsb", name="wt_sb")
                nc.scalar.copy(wt_sb[:, :, :], wt_ps[:, :, :])
                if DEBUG:
                    nc.sync.dma_start(out=dbg_wt[g, it], in_=wt_sb[:, :, :])
                yield

                # ---- b-MM: blogit[(b,i), j] = sum_d wtil[d,(b,i)] x[b][d, j] ----
                b4_ps = psum.tile([128, 8, 128], F32, tag="big", bufs=2, name="b4_ps")
                e4_sb = work.tile([128, 8, 128], BF16, tag="e4", name="e4_sb")
                eT_sb = work.tile([128, 8, 128], BF16, tag="eT", name="eT_sb")
                for jh in range(2):
                    for dc in range(2):
                        for b in range(GB):
                            nc.tensor.matmul(
                                b4_ps[32 * b : 32 * b + 32, 4 * jh : 4 * jh + 4, :],
                                wt_sb[:, dc, 32 * b : 32 * b + 32],
                                xd_t[b][:, dc, 512 * jh : 512 * jh + 512],
                                start=(dc == 0),
                                stop=(dc == 1),
                                tile_position=(0, 32 * b),
                                skip_group_check=True,
                            )
                    # softmax numerator with the squash scale folded in:
                    # e = exp(rn * b)  (rn constant along j -> same softmax)
                    nc.scalar.activation(
                        e4_sb[:, 4 * jh : 4 * jh + 4, :],
                        b4_ps[:, 4 * jh : 4 * jh + 4, :],
                        AF.Exp,
                        scale=rn4[:, :],
                    )
                # transposes issued after both exps: the DMA occupies the ACT
                # HWDGE queue, so tp(h0) must not sit between the two exps
                for jh in range(2):
                    nc.scalar.dma_start_transpose(
                        eT_sb[:, 4 * jh : 4 * jh + 4, :],
                        e4_sb[:, 4 * jh : 4 * jh + 4, :].rearrange("p c n -> p (c n)"),
                    )
                yield

                if DEBUG:
                    nc.sync.dma_start(out=dbg_e4[g, it], in_=e4_sb[:, :, :])
                zT_sb = work.tile([128, 8, GB], F32, tag="zT", name="zT_sb")
                rz_sb = work.tile([128, 8, GB], F32, tag="rz", name="rz_sb")
                cT_sb = work.tile([128, 8, 128], BF16, tag="cT", name="cT_sb")
                mul_eng = nc.gpsimd if USE_GS else nc.vector
                for h in range(2):
                    hc = slice(4 * h, 4 * h + 4)
                    nc.vector.tensor_reduce(
                        zT_sb[:, hc, :],
                        eT_sb[:, hc, :].rearrange("p c (b i) -> p c b i", b=GB),
                        axis=mybir.AxisListType.X,
                        op=ALU.add,
                    )
                    nc.vector.reciprocal(rz_sb[:, hc, :], zT_sb[:, hc, :])
                    mul_eng.tensor_tensor(
                        cT_sb[:, hc, :].rearrange("p c (b i) -> p c b i", b=GB),
                        eT_sb[:, hc, :].rearrange("p c (b i) -> p c b i", b=GB),
                        rz_sb[:, hc, :].unsqueeze(3).broadcast_to([128, GB, GB, NUM_CAPS]),
                        ALU.mult,
                    )
                if DEBUG:
                    nc.sync.dma_start(out=dbg_ct[g, it], in_=cT_sb[:, :, :])
                yield

        streams = [group_stream(g) for g in range(GROUPS)]
        alive = list(streams)
        while alive:
            keep = []
            for s in alive:
                try:
                    next(s)
                    keep.append(s)
                except StopIteration:
                    pass
            alive = keep

    nc.compile()
    return nc


def _host_inputs(x, W):
    import ml_dtypes

    bf16 = ml_dtypes.bfloat16
    x = np.ascontiguousarray(np.asarray(x, dtype=np.float32))
    W = np.ascontiguousarray(np.asarray(W, dtype=np.float32)).reshape(D_IN, IK)
    xT = np.ascontiguousarray(x.transpose(0, 2, 1)).astype(bf16)
    WT = np.ascontiguousarray(W.T).astype(bf16)
    x = x.astype(bf16)
    Wb = W.astype(bf16)
    p = np.arange(128)
    # maskT[p, ikc, col] = 1 iff capsule_of(ik = 128*ikc + p) == col % 32
    ikc = np.arange(8)
    col = np.arange(128)
    maskt = (
        (4 * ikc[None, :, None] + p[:, None, None] // DIM_CAPS) == col[None, None, :] % NUM_CAPS
    ).astype(bf16)
    sel = (p[:, None] % DIM_CAPS == np.arange(DIM_CAPS)[None, :]).astype(bf16)
    ident = np.eye(128, dtype=np.float32)
    return x, xT, Wb, WT, maskt, sel, ident


_prog_cache = {}


def _get_program():
    if "nc" not in _prog_cache:
        _prog_cache["nc"] = build_program()
    return _prog_cache["nc"]


def _in_maps(x, W):
    x, xT, Wb, WT, maskt, sel, ident = _host_inputs(x, W)
    in_maps = []
    for c in range(N_CORES):
        sl = slice(c * B_PER_CORE, (c + 1) * B_PER_CORE)
        in_maps.append(
            {
                "x_b": x[sl],
                "x_d": xT[sl],
                "w_a": Wb,
                "w_t": WT,
                "maskt": maskt,
                "sel": sel,
                "ident": ident,
            }
        )
    return in_maps


def _extract_out(res):
    """res -> [B_TOTAL, NUM_CAPS, DIM_CAPS]; out is [GROUPS, k, (4b,32i)]."""
    out = np.empty((B_TOTAL, NUM_CAPS, DIM_CAPS), np.float32)
    for c in range(N_CORES):
        o = np.asarray(res.results[c]["out"], np.float32)  # [GROUPS, 32, 128]
        o = o.transpose(0, 2, 1).reshape(B_PER_CORE, NUM_CAPS, DIM_CAPS)
        out[c * B_PER_CORE : (c + 1) * B_PER_CORE] = o
    return out


def kernel(x, W):
    in_maps = _in_maps(x, W)
    nc = _get_program()
    res = run_bass_kernel_spmd(nc, in_maps, core_ids=list(range(N_CORES)))
    return _extract_out(res)


# revision 42
# speedup vs baseline: 1.0648x; 1.0648x over previous
"""Trainium2 Bass kernel for the capsule-routing layer.

Math (derived from the reference):
  u_hat[b,i,j,k] = sum_d x[b,j,d] W[d, i*32+k]   (never materialized!)
  iter t: c = softmax_i(b_logits); s[i,k] = sum_j c[i,j] u_hat[i,j,k]
          o = s / sqrt(sum_k s^2 + eps); b_logits[i,j] = sum_k o[i,k] u_hat[i,j,k]
Substituting u_hat = x @ W everywhere:
  y[i,d]   = sum_j c[i,j] x[j,d]            (small matmul, K=1024)
  sT[ik,i] = sum_d W[d,ik] yT[d,i]          (W stationary -> s lands transposed)
  wtil[d,i]= sum_ik WT[ik,d] maskT*sT       (WT stationary -> wtil lands [d, i])
  b[i,j]   = sum_d wtil[d,i] x[j,d]         (small matmul, K=256)
  exp(rn*b) folds the squash scale into the softmax numerator (softmax of
  rn*b equals softmax-of-(o . u_hat)) since rn is constant along j.
This removes the 34-GFLOP u_hat product entirely (~7.6x FLOP reduction), and
the transposed-s / transposed-wtil orientations remove all DMA transposes
from the s -> wtil -> b chain (only the e-transpose remains).

The capsule norm (sum_k s^2) in transposed space is a partition reduction:
square on ACT, ones-matmul on PE (replicated to 32 rows), then 4 DVE 32x32
stream-transposes flip the [1,128] row into the [128,1] per-partition scalar
that exp(scale=rn) consumes. The whole norm branch runs concurrently with
the wtil/b matmuls.

Sharding: data-parallel, 8 batches per core; batches processed in groups of
4 stacked on SBUF partitions (partition p = 32*b + i).
"""

import numpy as np

try:
    import concourse.bass as bass
except ImportError:  # path fallback for bare environments
    import sys

    sys.path.insert(0, "/opt/trn_rl_repo")
    import concourse.bass as bass

from contextlib import ExitStack

import concourse.bacc as bacc
import concourse.tile as tile
from concourse import mybir
from concourse.bass_utils import run_bass_kernel_spmd

F32 = mybir.dt.float32
F32R = mybir.dt.float32r
BF16 = mybir.dt.bfloat16
AF = mybir.ActivationFunctionType
ALU = mybir.AluOpType

NUM_CAPS = 32
DIM_CAPS = 32
D_IN = 256  # feature dim (d)
N_IN = 1024  # input capsule count (j)
IK = NUM_CAPS * DIM_CAPS  # 1024 flattened (i,k)
B_TOTAL = 64
N_CORES = 8
B_PER_CORE = 8
GB = 4  # batches per partition-group
GROUPS = B_PER_CORE // GB  # 2
EPS = 1e-7
ROUTINGS = 3

import os as _os

DEBUG = _os.environ.get("K_DEBUG", "0") == "1"
USE_GS = _os.environ.get("K_GS", "1") == "1"  # softmax reduce/mult on GpSimd
NEWTON = int(_os.environ.get("K_NEWTON", "2"))


def build_program():
    nc = bacc.Bacc("TRN2", target_bir_lowering=False, debug=False)

    x_b = nc.declare_dram_parameter("x_b", [B_PER_CORE, N_IN, D_IN], BF16, isOutput=False)
    x_d = nc.declare_dram_parameter("x_d", [B_PER_CORE, D_IN, N_IN], BF16, isOutput=False)
    w_a = nc.declare_dram_parameter("w_a", [D_IN, IK], BF16, isOutput=False)
    w_t = nc.declare_dram_parameter("w_t", [IK, D_IN], BF16, isOutput=False)
    maskt_d = nc.declare_dram_parameter("maskt", [128, 8, 128], BF16, isOutput=False)
    sel_d = nc.declare_dram_parameter("sel", [128, DIM_CAPS], BF16, isOutput=False)
    ident_d = nc.declare_dram_parameter("ident", [128, 128], F32, isOutput=False)
    # transposed output: out[g, k, (b,i)]
    out_d = nc.declare_dram_parameter("out", [GROUPS, DIM_CAPS, 128], F32, isOutput=True)
    if DEBUG:
        dbg_m4t = nc.declare_dram_parameter("dbg_m4t", [GROUPS, 2, 128, 8, 128], BF16, isOutput=True)
        dbg_nrm = nc.declare_dram_parameter("dbg_nrm", [GROUPS, 2, 128, 1], F32, isOutput=True)
        dbg_wt = nc.declare_dram_parameter("dbg_wt", [GROUPS, 2, 128, 2, 128], BF16, isOutput=True)
        dbg_e4 = nc.declare_dram_parameter("dbg_e4", [GROUPS, 2, 128, 8, 128], BF16, isOutput=True)
        dbg_ct = nc.declare_dram_parameter("dbg_ct", [GROUPS, 2, 128, 8, 128], BF16, isOutput=True)

    with ExitStack() as ctx:
        tc = ctx.enter_context(tile.TileContext(nc))
        singles = ctx.enter_context(tc.tile_pool(name="singles", bufs=1))
        xpool = ctx.enter_context(tc.tile_pool(name="xpool", bufs=8))
        work = ctx.enter_context(tc.tile_pool(name="work", bufs=3))
        psum = ctx.enter_context(tc.tile_pool(name="ps", bufs=1, space="PSUM"))

        # ---- static tensors ----
        # All bulk transfers ride the SP (sync) HWDGE queue in need-order;
        # the Activation queue carries only tiny early statics. Bulk loads on
        # the ACT queue stall mid-kernel activations behind ring waits.
        ident_sb = singles.tile([128, 128], F32)
        nc.sync.dma_start(out=ident_sb[:, :], in_=ident_d[:, :])
        w_a_sb = singles.tile([128, 2, IK], BF16)  # [d%128, d//128, (ik)]
        nc.scalar.dma_start(out=w_a_sb[:, :, :], in_=w_a[:, :].rearrange("(c p) n -> p c n", p=128))
        w_t_sb = singles.tile([128, 8, D_IN], BF16)  # [(ik)%128, (ik)//128, d]
        nc.scalar.dma_start(out=w_t_sb[:, :, :], in_=w_t[:, :].rearrange("(c p) n -> p c n", p=128))
        maskt_sb = singles.tile([128, 8, 128], BF16)
        nc.scalar.dma_start(out=maskt_sb[:, :, :], in_=maskt_d[:, :, :])
        cu_sb = singles.tile([128, NUM_CAPS], BF16)
        nc.vector.memset(cu_sb[:, :], 1.0 / NUM_CAPS)
        ones_sb = singles.tile([128, NUM_CAPS], F32)
        nc.vector.memset(ones_sb[:, :], 1.0)
        magic_sb = singles.tile([128, 1], mybir.dt.int32)
        nc.vector.memset(magic_sb[:, :], 0x5F3759DF)
        one_i_sb = singles.tile([128, 1], mybir.dt.int32)
        nc.vector.memset(one_i_sb[:, :], 1)
        # sel32[p, m] = 1 iff p % 32 == m: partition-strided k-group reducer
        sel_sb = singles.tile([128, DIM_CAPS], BF16)
        nc.scalar.dma_start(out=sel_sb[:, :], in_=sel_d[:, :])

        # ---- HAM warm-up: keep the PE busy while inputs stream in so the
        # clock gate is at 8/8 when the first real matmul issues ----
        warm_ps = psum.tile([32, 128], F32, tag="nrm", bufs=2, name="warm_ps")
        for _ in range(36):
            nc.tensor.matmul(
                warm_ps[:, :],
                ident_sb[:, 0:32],
                ident_sb[:, :],
                start=True,
                stop=True,
                skip_group_check=True,
            )

        def rsqrt_dve(a_ap, tagp):
            """1/sqrt(a): DVE quake bit-trick + Newton iterations."""
            t_i = work.tile([128, 1], mybir.dt.int32, tag=tagp + "i", name="nr_i")
            nc.vector.tensor_tensor(
                t_i[:, :], a_ap.bitcast(mybir.dt.int32), one_i_sb[:, :], ALU.logical_shift_right
            )
            r = work.tile([128, 1], F32, tag=tagp + "r", name="nr_r")
            nc.vector.tensor_tensor(
                r[:, :].bitcast(mybir.dt.int32), magic_sb[:, :], t_i[:, :], ALU.subtract
            )
            t2 = work.tile([128, 1], F32, tag=tagp + "t", name="nr_t")
            for _ in range(NEWTON):
                nc.vector.tensor_mul(t2[:, :], a_ap, r[:, :])
                nc.vector.tensor_mul(t2[:, :], t2[:, :], r[:, :])
                nc.vector.tensor_scalar(t2[:, :], t2[:, :], -0.5, 1.5, ALU.mult, ALU.add)
                nc.vector.tensor_mul(r[:, :], r[:, :], t2[:, :])
            return r

        def rsqrt_row(a_ap, tagp):
            """1/sqrt(a) for [32, 128] row tiles (quake + Newton)."""
            t_i = work.tile([32, 128], mybir.dt.int32, tag=tagp + "ri", name="nw_i")
            nc.vector.tensor_tensor(
                t_i[:, :],
                a_ap.bitcast(mybir.dt.int32),
                one_i_sb[0:32, 0:1].broadcast_to([32, 128]),
                ALU.logical_shift_right,
            )
            r = work.tile([32, 128], F32, tag=tagp + "rr", name="nw_r")
            nc.vector.tensor_tensor(
                r[:, :].bitcast(mybir.dt.int32),
                magic_sb[0:32, 0:1].broadcast_to([32, 128]),
                t_i[:, :],
                ALU.subtract,
            )
            t2 = work.tile([32, 128], F32, tag=tagp + "rt", name="nw_t")
            for _ in range(NEWTON):
                nc.vector.tensor_mul(t2[:, :], a_ap, r[:, :])
                nc.vector.tensor_mul(t2[:, :], t2[:, :], r[:, :])
                nc.vector.tensor_scalar(t2[:, :], t2[:, :], -0.5, 1.5, ALU.mult, ALU.add)
                nc.vector.tensor_mul(r[:, :], r[:, :], t2[:, :])
            return r

        def group_stream(g):
            # ---- load this group's x in both layouts ----
            xb_t = []
            xd_t = []
            for b in range(GB):
                bb = g * GB + b
                xb = xpool.tile([128, 8, D_IN], BF16, tag="xb", name=f"xb{bb}")
                nc.sync.dma_start(out=xb[:, :, :], in_=x_b[bb].rearrange("(c p) n -> p c n", p=128))
                xb_t.append(xb)
            for b in range(GB):
                bb = g * GB + b
                xd = xpool.tile([128, 2, N_IN], BF16, tag="xd", name=f"xd{bb}")
                nc.sync.dma_start(out=xd[:, :, :], in_=x_d[bb].rearrange("(c p) n -> p c n", p=128))
                xd_t.append(xd)
            yield

            cT_sb = None  # [j%128, j//128, (4b,32i)] softmax'd coupling coeffs
            for it in range(ROUTINGS):
                last = it == ROUTINGS - 1

                # ---- y-MM: y[b,i,d] = sum_j c[b,i,j] x[b,j,d] ----
                y4_ps = psum.tile([128, 2, 128], F32, tag="m32", bufs=2, name="y4_ps")
                for jc in range(8):
                    for b in range(GB):
                        lhsT = cu_sb[:, :] if it == 0 else cT_sb[:, jc, 32 * b : 32 * b + 32]
                        nc.tensor.matmul(
                            y4_ps[32 * b : 32 * b + 32, :, :].rearrange("p c n -> p (c n)"),
                            lhsT,
                            xb_t[b][:, jc, :],
                            start=(jc == 0),
                            stop=(jc == 7),
                            tile_position=(0, 32 * b),
                            skip_group_check=True,
                        )
                yield

                # evacuate + transpose y -> [d, (4b,32i)]
                y4_sb = work.tile([128, D_IN], F32, tag="y4sb", name="y4_sb")
                nc.scalar.copy(y4_sb[:, :], y4_ps[:, :, :].rearrange("p c n -> p (c n)"))
                yT_ps = psum.tile([128, 2, 128], F32, tag="m32", bufs=2, name="yT_ps")
                for t in range(2):
                    nc.tensor.transpose(yT_ps[:, t, :], y4_sb[:, 128 * t : 128 * t + 128], ident_sb[:, :])
                yT_sb = work.tile([128, 2, 128], BF16, tag="yTsb", name="yT_sb")
                nc.scalar.copy(yT_sb[:, :, :], yT_ps[:, :, :])
                yield

                if last:
                    # ---- last iter, fully in transposed space ----
                    # sT-MM + mask as in the main loop, then a selector-matmul
                    # (sel32[p,m]=1 iff p%32==m) does the per-k compaction as a
                    # partition-strided reduce on the PE:
                    #   scT[k, (b,i)] = sum_ikc sum_{p%32==k} m4T[p, ikc, (b,i)]
                    sT_ps = psum.tile([128, 8, 128], F32, tag="big", bufs=2, name="sT_ps")
                    m4t_sb = work.tile([128, 8, 128], BF16, tag="m4t", name="m4t_sb")
                    for ikc in range(8):
                        for dc in range(2):
                            nc.tensor.matmul(
                                sT_ps[:, ikc, :],
                                w_a_sb[:, dc, 128 * ikc : 128 * ikc + 128],
                                yT_sb[:, dc, :],
                                start=(dc == 0),
                                stop=(dc == 1),
                                skip_group_check=True,
                            )
                    for h in range(2):
                        hs = slice(4 * h, 4 * h + 4)
                        nc.vector.tensor_mul(m4t_sb[:, hs, :], sT_ps[:, hs, :], maskt_sb[:, hs, :])
                    scT_ps = psum.tile([32, 128], F32, tag="nrm", bufs=2, name="scT_ps")
                    for ikc in range(8):
                        nc.tensor.matmul(
                            scT_ps[:, :],
                            sel_sb[:, :],
                            m4t_sb[:, ikc, :],
                            start=(ikc == 0),
                            stop=(ikc == 7),
                            skip_group_check=True,
                        )
                    # norm rows: nr[*, (b,i)] = sum_k scT^2, replicated to 32 rows
                    sq_sb = work.tile([32, 128], F32, tag="lsq", name="lsq")
                    nc.scalar.activation(sq_sb[:, :], scT_ps[:, :], AF.Square)
                    nrw_ps = psum.tile([32, 128], F32, tag="nrm", bufs=2, name="nrw_ps")
                    nc.tensor.matmul(
                        nrw_ps[:, :],
                        ones_sb[0:32, :],
                        sq_sb[:, :],
                        start=True,
                        stop=True,
                        skip_group_check=True,
                    )
                    nre = work.tile([32, 128], F32, tag="lne", name="lne")
                    nc.vector.tensor_scalar(nre[:, :], nrw_ps[:, :], EPS, None, ALU.add)
                    rnr = rsqrt_row(nre[:, :], "lst")
                    o_outT = work.tile([32, 128], F32, tag="ooT", name="o_outT")
                    nc.vector.tensor_tensor(o_outT[:, :], scT_ps[:, :], rnr[:, :], ALU.mult)
                    nc.scalar.dma_start(out=out_d[g], in_=o_outT[:, :])
                    return

                # ---- sT-MM: sT[(ik),(b,i)] = sum_d W[d,(ik)] yT[d,(b,i)] ----
                # W chunks are the stationary operand (bf16 -> FWL), s lands
                # pre-transposed so no DMA transpose is needed before wtil.
                sT_ps = psum.tile([128, 8, 128], F32, tag="big", bufs=2, name="sT_ps")
                m4t_sb = work.tile([128, 8, 128], BF16, tag="m4t", name="m4t_sb")
                sqt_sb = work.tile([128, 8, 128], F32, tag="sqt", name="sqt_sb")
                for ikc in range(8):
                    for dc in range(2):
                        nc.tensor.matmul(
                            sT_ps[:, ikc, :],
                            w_a_sb[:, dc, 128 * ikc : 128 * ikc + 128],
                            yT_sb[:, dc, :],
                            start=(dc == 0),
                            stop=(dc == 1),
                            skip_group_check=True,
                        )
                for h in range(2):
                    hs = slice(4 * h, 4 * h + 4)
                    nc.vector.tensor_mul(m4t_sb[:, hs, :], sT_ps[:, hs, :], maskt_sb[:, hs, :])
                    nc.scalar.activation(sqt_sb[:, hs, :], m4t_sb[:, hs, :], AF.Square)
                yield

                # ---- capsule norm: partition-reduce sum of squares via ones-MM ----
                nrm_ps = psum.tile([32, 128], F32, tag="nrm", bufs=2, name="nrm_ps")
                for ikc in range(8):
                    nc.tensor.matmul(
                        nrm_ps[:, :],
                        ones_sb[:, :],
                        sqt_sb[:, ikc, :],
                        start=(ikc == 0),
                        stop=(ikc == 7),
                        skip_group_check=True,
                    )
                # flip [1,128] norm row -> [128,1] per-partition scalar (DVE 32x32)
                nflip = work.tile([128, 32], F32, tag="nflip", name="nflip")
                for q in range(4):
                    nc.vector.transpose(nflip[32 * q : 32 * q + 32, :], nrm_ps[0:32, 32 * q : 32 * q + 32])
                nsq4 = work.tile([128, 1], F32, tag="nsq4", name="nsq4")
                nc.vector.tensor_scalar(nsq4[:, :], nflip[:, 0:1], EPS, None, ALU.add)
                rn4 = rsqrt_dve(nsq4[:, :], "mid")
                if DEBUG:
                    nc.sync.dma_start(out=dbg_m4t[g, it], in_=m4t_sb[:, :, :])
                    nc.sync.dma_start(out=dbg_nrm[g, it], in_=nsq4[:, :])
                yield

                # ---- wtil-MM: wtil[d,(b,i)] = sum_ik WT[(ik),d] m4T[(ik),(b,i)] ----
                # NB: accumulation must be contiguous per region — start=True
                # clears has_written for the whole bank, so interleaving two
                # regions' groups in one bank corrupts the first region.
                wt_ps = psum.tile([128, 2, 128], F32, tag="m32", bufs=2, name="wt_ps")
                for dh in range(2):
                    for ikc in range(8):
                        nc.tensor.matmul(
                            wt_ps[:, dh, :],
                            w_t_sb[:, ikc, 128 * dh : 128 * dh + 128],
                            m4t_sb[:, ikc, :],
                            start=(ikc == 0),
                            stop=(ikc == 7),
                            skip_group_check=True,
                        )
                wt_sb = work.tile([128, 2, 128], BF16, tag="wtsb", name="wt_sb")
                nc.scalar.copy(wt_sb[:, :, :], wt_ps[:, :, :])
                if DEBUG:
                    nc.sync.dma_start(out=dbg_wt[g, it], in_=wt_sb[:, :, :])
                yield

                # ---- b-MM: blogit[(b,i), j] = sum_d wtil[d,(b,i)] x[b][d, j] ----
                b4_ps = psum.tile([128, 8, 128], F32, tag="big", bufs=2, name="b4_ps")
                e4_sb = work.tile([128, 8, 128], BF16, tag="e4", name="e4_sb")
                eT_sb = work.tile([128, 8, 128], BF16, tag="eT", name="eT_sb")
                for jh in range(2):
                    for dc in range(2):
                        for b in range(GB):
                            nc.tensor.matmul(
                                b4_ps[32 * b : 32 * b + 32, 4 * jh : 4 * jh + 4, :],
                                wt_sb[:, dc, 32 * b : 32 * b + 32],
                                xd_t[b][:, dc, 512 * jh : 512 * jh + 512],
                                start=(dc == 0),
                                stop=(dc == 1),
                                tile_position=(0, 32 * b),
                                skip_group_check=True,
                            )
                    # softmax numerator with the squash scale folded in:
                    # e = exp(rn * b)  (rn constant along j -> same softmax)
                    nc.scalar.activation(
                        e4_sb[:, 4 * jh : 4 * jh + 4, :],
                        b4_ps[:, 4 * jh : 4 * jh + 4, :],
                        AF.Exp,
                        scale=rn4[:, :],
                    )
                # transposes issued after both exps on the sync queue (free
                # once input loads drain); keeping them off the ACT queue
                # avoids stalling later activations behind the transfer
                for jh in range(2):
                    nc.sync.dma_start_transpose(
                        eT_sb[:, 4 * jh : 4 * jh + 4, :],
                        e4_sb[:, 4 * jh : 4 * jh + 4, :].rearrange("p c n -> p (c n)"),
                    )
                yield

                if DEBUG:
                    nc.sync.dma_start(out=dbg_e4[g, it], in_=e4_sb[:, :, :])
                zT_sb = work.tile([128, 8, GB], F32, tag="zT", name="zT_sb")
                rz_sb = work.tile([128, 8, GB], F32, tag="rz", name="rz_sb")
                cT_sb = work.tile([128, 8, 128], BF16, tag="cT", name="cT_sb")
                mul_eng = nc.gpsimd if USE_GS else nc.vector
                for h in range(2):
                    hc = slice(4 * h, 4 * h + 4)
                    nc.vector.tensor_reduce(
                        zT_sb[:, hc, :],
                        eT_sb[:, hc, :].rearrange("p c (b i) -> p c b i", b=GB),
                        axis=mybir.AxisListType.X,
                        op=ALU.add,
                    )
                    nc.vector.reciprocal(rz_sb[:, hc, :], zT_sb[:, hc, :])
                    mul_eng.tensor_tensor(
                        cT_sb[:, hc, :].rearrange("p c (b i) -> p c b i", b=GB),
                        eT_sb[:, hc, :].rearrange("p c (b i) -> p c b i", b=GB),
                        rz_sb[:, hc, :].unsqueeze(3).broadcast_to([128, GB, GB, NUM_CAPS]),
                        ALU.mult,
                    )
                if DEBUG:
                    nc.sync.dma_start(out=dbg_ct[g, it], in_=cT_sb[:, :, :])
                yield

        streams = [group_stream(g) for g in range(GROUPS)]
        alive = list(streams)
        while alive:
            keep = []
            for s in alive:
                try:
                    next(s)
                    keep.append(s)
                except StopIteration:
                    pass
            alive = keep

    nc.compile()
    return nc


def _host_inputs(x, W):
    import ml_dtypes

    bf16 = ml_dtypes.bfloat16
    x = np.ascontiguousarray(np.asarray(x, dtype=np.float32))
    W = np.ascontiguousarray(np.asarray(W, dtype=np.float32)).reshape(D_IN, IK)
    xT = np.ascontiguousarray(x.transpose(0, 2, 1)).astype(bf16)
    WT = np.ascontiguousarray(W.T).astype(bf16)
    x = x.astype(bf16)
    Wb = W.astype(bf16)
    p = np.arange(128)
    # maskT[p, ikc, col] = 1 iff capsule_of(ik = 128*ikc + p) == col % 32
    ikc = np.arange(8)
    col = np.arange(128)
    maskt = (
        (4 * ikc[None, :, None] + p[:, None, None] // DIM_CAPS) == col[None, None, :] % NUM_CAPS
    ).astype(bf16)
    sel = (p[:, None] % DIM_CAPS == np.arange(DIM_CAPS)[None, :]).astype(bf16)
    ident = np.eye(128, dtype=np.float32)
    return x, xT, Wb, WT, maskt, sel, ident


_prog_cache = {}


def _get_program():
    if "nc" not in _prog_cache:
        _prog_cache["nc"] = build_program()
    return _prog_cache["nc"]


def _in_maps(x, W):
    x, xT, Wb, WT, maskt, sel, ident = _host_inputs(x, W)
    in_maps = []
    for c in range(N_CORES):
        sl = slice(c * B_PER_CORE, (c + 1) * B_PER_CORE)
        in_maps.append(
            {
                "x_b": x[sl],
                "x_d": xT[sl],
                "w_a": Wb,
                "w_t": WT,
                "maskt": maskt,
                "sel": sel,
                "ident": ident,
            }
        )
    return in_maps


def _extract_out(res):
    """res -> [B_TOTAL, NUM_CAPS, DIM_CAPS]; out is [GROUPS, k, (4b,32i)]."""
    out = np.empty((B_TOTAL, NUM_CAPS, DIM_CAPS), np.float32)
    for c in range(N_CORES):
        o = np.asarray(res.results[c]["out"], np.float32)  # [GROUPS, 32, 128]
        o = o.transpose(0, 2, 1).reshape(B_PER_CORE, NUM_CAPS, DIM_CAPS)
        out[c * B_PER_CORE : (c + 1) * B_PER_CORE] = o
    return out


def kernel(x, W):
    in_maps = _in_maps(x, W)
    nc = _get_program()
    res = run_bass_kernel_spmd(nc, in_maps, core_ids=list(range(N_CORES)))
    return _extract_out(res)
